# revision 20
# baseline (speedup 1.0000x reference)
"""BitLinearLRLS fused fp8-DoubleRow kernel for 8 Trainium2 NeuronCores.

Math (see reference):
    w_q       = clip(round(weight / 0.5), -1, 1)            # ternary, RNE ties
    x_mean    = mean(x, axis=(0,1))                         # [in]
    scale_eff = scale + lrls_A @ (lrls_B @ x_mean)          # [out]
    y         = x @ (w_q * scale_eff[:, None]).T

Key identity: y = (x @ w_q.T) * scale_eff[None, :] — the big matmul does not
depend on scale_eff, so the data-dependent scale is a per-output-row epilogue.

Precision: x is cast to fp8 e4m3 on the host (RNE); w_q in {-1,0,1} is exact
in fp8. Measured end-to-end max rel err on the seeded inputs: 1.68e-2
(budget 2e-2). The win: fp8 with perf_mode=DoubleRow packs two contraction
rows per PE cell — a 256-deep k-chunk per 512-column matmul, ~1.5x the
fp32r/bf16 row rate.

Sharding: data-parallel over tokens, tok/8 = 2048 per core. x^T (fp8) is
SBUF-resident (64 KiB/partition), read from HBM exactly once. Weights stream
in 16 slabs of 256 out-features, quantized on the DVE (exact int32-RNE cast
chain), laid out as [128, 2, osize] chunk-pair tiles for DoubleRow.

Engine plan (queues are FIFO — a blocked op stalls everything behind it):
  PE     : 2048 [256x128x512] fp8 DoubleRow matmuls, nothing else
  DVE    : w quantize (mult+max / min+i32cast / i32->fp8 copy), token sums
           (fp8 reduce), LRLS matvec chain (emitted after slab 4's quantize
           so the cc_out wait never starves later quantizes)
  ACT    : epilogues only. Slabs 0..SWITCH-1: unscaled PSUM->SBUF copies
           (frees banks with no scale_eff dependency — covers the AllReduce
           latency), with the scale passes emitted in a burst at the end of
           slab SWITCH-1. Slabs >= SWITCH: direct scaled epilogue.
  sync   : x pair-tile loads, cc_in copy, all y stores
  gpsimd : w quad-chunk loads (SWDGE), collective, LRLS small loads

Quantization is exact vs the reference:
    w_q = fp8(int32_cast_rne(clamp(2w, -1.25, 1.25)))
(the DVE float->int cast is round-to-nearest-even, matching round(); -1/0/1
are exact in fp8 e4m3).
"""

import numpy as np
import ml_dtypes

import concourse.bass as bass
import concourse.tile as tile
from concourse import bacc, mybir
from concourse.bass_utils import run_bass_kernel_spmd

F32 = mybir.dt.float32
F8 = mybir.dt.float8e4
I32 = mybir.dt.int32
ALU = mybir.AluOpType
ACTF = mybir.ActivationFunctionType
DR = mybir.MatmulPerfMode.DoubleRow


class Cfg:
    def __init__(self, tok=16384, din=4096, dout=4096, r=16,
                 tsh=None, oslab=256, tblk=512, ncores=8, switch=4):
        self.ncores = ncores
        self.tok = tok            # total tokens (B*S)
        self.din = din
        self.dout = dout
        self.r = r
        self.tsh = tsh or tok // ncores   # tokens per core
        self.oslab = oslab        # output features per weight slab
        self.tblk = tblk          # moving-operand tile (tokens)
        self.kc = din // 128      # 128-row contraction chunks
        self.kc2 = din // 256     # 256-row DoubleRow chunk pairs
        self.kc4 = din // 512     # 512-row w-load quads
        self.oc = dout // 128     # output 128-chunks (scale_eff columns)
        self.nslab = dout // oslab
        self.ntblk = self.tsh // tblk
        self.nos = oslab // 128   # psum groups per t-block (out dim)
        self.switch = switch      # slabs using the staged (deferred-scale) path
        self.lrls_slab = switch   # emission slab for LRLS + deferred burst
        self.bchunk = min(512, din // 4)   # LRLS B matvec chunk
        assert self.nos * self.ntblk <= 8
        assert din % self.bchunk == 0


def build(cfg: Cfg, compile=True):
    nc = bacc.Bacc("TRN2", target_bir_lowering=False, debug=False,
                   enable_asserts=True, num_devices=cfg.ncores)

    x8T = nc.dram_tensor("x8T", [cfg.din, cfg.tsh], F8,
                         kind="ExternalInput").ap()
    wT = nc.dram_tensor("wT", [cfg.din, cfg.dout], F32,
                        kind="ExternalInput").ap()
    scale_pc = nc.dram_tensor("scale_pc", [128, cfg.oc], F32,
                              kind="ExternalInput").ap()
    # b_pk[r, p*kc + k] = B[r, k*128 + p]  (matches sums' [p, k] flat order)
    b_pk = nc.dram_tensor("b_pk", [cfg.r, cfg.din], F32,
                          kind="ExternalInput").ap()
    # a_p[p, j*r + rr] = A[j*128 + p, rr]
    a_p = nc.dram_tensor("a_p", [128, cfg.oc * cfg.r], F32,
                         kind="ExternalInput").ap()
    yT = nc.dram_tensor("yT", [cfg.dout, cfg.tsh], F32,
                        kind="ExternalOutput").ap()

    with tile.TileContext(nc) as tc:
        with tc.tile_pool(name="keep", bufs=1) as keep, \
             tc.tile_pool(name="cdram", bufs=1, space="DRAM") as cdram, \
             tc.tile_pool(name="xp", bufs=cfg.kc2) as xpp, \
             tc.tile_pool(name="wst", bufs=4) as wst, \
             tc.tile_pool(name="qa", bufs=2) as qap, \
             tc.tile_pool(name="qb", bufs=2) as qbp, \
             tc.tile_pool(name="wq", bufs=2 * cfg.kc2) as wqp, \
             tc.tile_pool(name="stage", bufs=cfg.switch * 8) as stp, \
             tc.tile_pool(name="yt", bufs=4) as ytp, \
             tc.tile_pool(name="lr", bufs=1) as lr, \
             tc.tile_pool(name="lr2", bufs=1) as lr2:

            sums = keep.tile([128, cfg.kc], F32)
            scale_eff = keep.tile([128, cfg.oc], F32)
            trash = keep.tile([128, cfg.tsh], F8)

            cc_in = cdram.tile([128, cfg.kc], F32)
            cc_out = cdram.tile([128, cfg.kc], F32)
            v_d = cdram.tile([1, cfg.r], F32)

            # ---- x: load fp8 x^T once, resident as chunk-pair tiles.
            # Token sums ride ACT (idle until the first epilogue) as
            # Copy-with-accum_out self-reduces, keeping the DVE free for the
            # weight quantize cadence the PE is fed by. ----
            xp = []
            for c in range(cfg.kc2):
                xt = xpp.tile([128, 2, cfg.tsh], F8, name="xt")
                nc.sync.dma_start(
                    out=xt,
                    in_=x8T[c * 256:(c + 1) * 256, :]
                    .rearrange("(two p) t -> p two t", p=128))
                xp.append(xt)
            for ib in range(cfg.kc // 2):
                nc.scalar.activation(
                    out=trash, in_=xp[ib // 2][:, ib % 2, :],
                    func=ACTF.Copy, accum_out=sums[:, ib:ib + 1])

            def emit_lrls():
                # scale_eff = scale + A @ (B @ sum_x) / tok, pure DVE epilogue
                nchunk = cfg.din // cfg.bchunk
                vparts = lr.tile([cfg.r, nchunk], F32)
                for c in range(nchunk):
                    xbc = lr2.tile([cfg.r, cfg.bchunk], F32, name="xbc")
                    nc.gpsimd.dma_start(
                        out=xbc,
                        in_=bass.AP(tensor=cc_out.tensor,
                                    offset=cc_out.offset + c * cfg.bchunk,
                                    ap=[[0, cfg.r], [1, cfg.bchunk]]))
                    bsb = lr2.tile([cfg.r, cfg.bchunk], F32, name="bsb")
                    nc.gpsimd.dma_start(
                        out=bsb,
                        in_=b_pk[:, c * cfg.bchunk:(c + 1) * cfg.bchunk])
                    nc.vector.tensor_tensor(out=xbc, in0=bsb, in1=xbc,
                                            op=ALU.mult)
                    nc.vector.reduce_sum(out=vparts[:, c:c + 1], in_=xbc,
                                         axis=mybir.AxisListType.X)
                vsb = lr.tile([cfg.r, 1], F32)
                nc.vector.reduce_sum(out=vsb, in_=vparts,
                                     axis=mybir.AxisListType.X)
                nc.gpsimd.dma_start(out=v_d.rearrange("one r -> r one"),
                                    in_=vsb)

                # corr[p, j] = sum_rr a_p[p, j*r+rr] * v[rr]
                vb16 = lr.tile([128, cfg.r], F32)
                nc.gpsimd.dma_start(
                    out=vb16,
                    in_=bass.AP(tensor=v_d.tensor, offset=v_d.offset,
                                ap=[[0, 128], [1, cfg.r]]))
                ap_sb = lr.tile([128, cfg.oc, cfg.r], F32)
                nc.gpsimd.dma_start(
                    out=ap_sb,
                    in_=a_p.rearrange("p (j rr) -> p j rr", rr=cfg.r))
                am = lr.tile([128, cfg.oc, cfg.r], F32)
                for j in range(cfg.oc):
                    nc.vector.tensor_tensor(out=am[:, j, :],
                                            in0=ap_sb[:, j, :],
                                            in1=vb16, op=ALU.mult)
                corr = lr.tile([128, cfg.oc], F32)
                nc.vector.reduce_sum(out=corr, in_=am,
                                     axis=mybir.AxisListType.X)
                sc_sb = lr.tile([128, cfg.oc], F32)
                nc.gpsimd.dma_start(out=sc_sb, in_=scale_pc)
                nc.vector.tensor_scalar(out=scale_eff, in0=corr,
                                        scalar1=1.0 / cfg.tok, scalar2=None,
                                        op0=ALU.mult)
                nc.vector.tensor_tensor(out=scale_eff, in0=scale_eff,
                                        in1=sc_sb, op=ALU.add)

            deferred = []   # (stage_tile, oi, tb) awaiting scale_eff

            with tc.tile_pool(name="ps", bufs=8, space="PSUM") as psp:
                for s in range(cfg.nslab):
                    ostart = s * cfg.oslab

                    # ---- w slab load (512-row quads) + quantize (pairs) ----
                    wq_tiles = []
                    for kb in range(cfg.kc4):
                        wr = wst.tile([128, 4, cfg.oslab], F32, name="wr")
                        nc.gpsimd.dma_start(
                            out=wr,
                            in_=wT[kb * 512:(kb + 1) * 512,
                                   ostart:ostart + cfg.oslab]
                            .rearrange("(four p) o -> p four o", p=128))
                        ga = qap.tile([128, 4, cfg.oslab], F32, name="ga")
                        nc.vector.tensor_scalar(
                            out=ga, in0=wr, scalar1=2.0, scalar2=-1.25,
                            op0=ALU.mult, op1=ALU.max)
                        gb = qbp.tile([128, 4, cfg.oslab], I32, name="gb")
                        nc.vector.tensor_scalar(
                            out=gb, in0=ga, scalar1=1.25, scalar2=None,
                            op0=ALU.min)
                        for half in range(2):
                            wq_t = wqp.tile([128, 2, cfg.oslab], F8,
                                            name="wq_t")
                            nc.vector.tensor_copy(
                                out=wq_t, in_=gb[:, 2 * half:2 * half + 2, :])
                            wq_tiles.append(wq_t)
                        if s == 1:
                            # second half of the token sums, woven through
                            # the DVE stream once all x pairs have landed
                            for i in range(2):
                                ib = cfg.kc // 2 + 2 * kb + i
                                nc.vector.reduce_sum(
                                    out=sums[:, ib:ib + 1],
                                    in_=xp[ib // 2][:, ib % 2, :],
                                    axis=mybir.AxisListType.X)
                    if s == 1:
                        nc.sync.dma_start(out=cc_in, in_=sums)
                        nc.gpsimd.collective_compute(
                            "AllReduce", ALU.add,
                            replica_groups=[list(range(cfg.ncores))],
                            ins=[cc_in.opt()], outs=[cc_out.opt()],
                        )
                    if s == cfg.lrls_slab:
                        # Tile deps are program-order: scale_eff's producers
                        # must be emitted before any consumer
                        emit_lrls()

                    # ---- matmuls: 256-deep DoubleRow chunks ----
                    pst = [[psp.tile([128, cfg.tblk], F32, name="pst")
                            for _ in range(cfg.ntblk)]
                           for _ in range(cfg.nos)]
                    for c in range(cfg.kc2):
                        for os_ in range(cfg.nos):
                            lhs = wq_tiles[c][:, :,
                                              os_ * 128:(os_ + 1) * 128]
                            for tb in range(cfg.ntblk):
                                nc.tensor.matmul(
                                    pst[os_][tb],
                                    lhsT=lhs,
                                    rhs=xp[c][:, :,
                                              tb * cfg.tblk:(tb + 1) * cfg.tblk],
                                    start=(c == 0), stop=(c == cfg.kc2 - 1),
                                    perf_mode=DR,
                                )

                    # ---- epilogues ----
                    for os_ in range(cfg.nos):
                        oi = ostart // 128 + os_
                        for tb in range(cfg.ntblk):
                            if s < cfg.switch:
                                # unscaled copy frees the bank with no
                                # scale_eff dependency
                                st = stp.tile([128, cfg.tblk], F32,
                                              name="st")
                                nc.scalar.activation(
                                    out=st, in_=pst[os_][tb], func=ACTF.Copy)
                                deferred.append((st, oi, tb))
                            else:
                                ytt = ytp.tile([128, cfg.tblk], F32,
                                               name="ytt")
                                nc.scalar.activation(
                                    out=ytt, in_=pst[os_][tb],
                                    func=ACTF.Copy,
                                    scale=scale_eff[:, oi:oi + 1])
                                nc.scalar.dma_start(
                                    out=yT[oi * 128:(oi + 1) * 128,
                                           tb * cfg.tblk:(tb + 1) * cfg.tblk],
                                    in_=ytt)

                    if s == cfg.lrls_slab:
                        # deferred scale+stores ride DVE (in-place on the
                        # stage tiles) + sync-queue stores: the scalar queue
                        # stays clear so later slabs' direct epilogues keep
                        # freeing PSUM banks promptly
                        for st, oi, tb in deferred:
                            nc.vector.tensor_tensor(
                                out=st, in0=st,
                                in1=scale_eff[:, oi:oi + 1]
                                .broadcast_to((128, cfg.tblk)),
                                op=ALU.mult)
                            nc.sync.dma_start(
                                out=yT[oi * 128:(oi + 1) * 128,
                                       tb * cfg.tblk:(tb + 1) * cfg.tblk],
                                in_=st)
                        deferred = []

    if compile:
        nc.compile()
    return nc


def prep_inputs(cfg: Cfg, x, weight, scale, lrls_A, lrls_B):
    """Host-side sharding/layout marshalling + the fp8 input cast."""
    x_flat = np.ascontiguousarray(x.reshape(cfg.tok, cfg.din))
    x8 = x_flat.astype(ml_dtypes.float8_e4m3)         # RNE
    x8T_full = np.ascontiguousarray(x8.T)             # [din, tok] fp8
    wT = np.ascontiguousarray(weight.T)               # [din, dout]
    b_pk = np.ascontiguousarray(
        lrls_B.reshape(cfg.r, cfg.kc, 128).transpose(0, 2, 1).reshape(
            cfg.r, cfg.din))
    a_p = np.ascontiguousarray(
        lrls_A.reshape(cfg.oc, 128, cfg.r).transpose(1, 0, 2).reshape(
            128, cfg.oc * cfg.r))
    scale_pc = np.ascontiguousarray(scale.reshape(cfg.oc, 128).T)

    in_maps = []
    for c in range(cfg.ncores):
        x8T_c = np.ascontiguousarray(
            x8T_full[:, c * cfg.tsh:(c + 1) * cfg.tsh])
        in_maps.append({"x8T": x8T_c, "wT": wT, "scale_pc": scale_pc,
                        "b_pk": b_pk, "a_p": a_p})
    return in_maps


def assemble_output(cfg: Cfg, results, out_shape):
    y_flat = np.empty((cfg.tok, cfg.dout), np.float32)
    for c in range(cfg.ncores):
        y_flat[c * cfg.tsh:(c + 1) * cfg.tsh, :] = results[c]["yT"].T
    return y_flat.reshape(out_shape)


_NC_CACHE = {}


def run(cfg: Cfg, x, weight, scale, lrls_A, lrls_B, out_shape, **run_kwargs):
    key = (cfg.tok, cfg.din, cfg.dout, cfg.tsh, cfg.oslab, cfg.tblk,
           cfg.switch)
    if key not in _NC_CACHE:
        _NC_CACHE[key] = build(cfg)
    nc = _NC_CACHE[key]
    in_maps = prep_inputs(cfg, x, weight, scale, lrls_A, lrls_B)
    res = run_bass_kernel_spmd(nc, in_maps, core_ids=list(range(cfg.ncores)),
                               **run_kwargs)
    y = assemble_output(cfg, res.results, out_shape)
    return y, res


def kernel(x, weight, threshold, scale, lrls_A, lrls_B):
    # threshold input is unused: the reference hardcodes THRESH=0.5
    # (TrainState.threshold() at step 0), so the ternary cut sits at |w|=0.25.
    cfg = Cfg()
    x = np.asarray(x, np.float32)
    y, _ = run(cfg, x, np.asarray(weight, np.float32),
               np.asarray(scale, np.float32), np.asarray(lrls_A, np.float32),
               np.asarray(lrls_B, np.float32),
               out_shape=(x.shape[0], x.shape[1], np.asarray(weight).shape[0]))
    return y.astype(np.float32)


# revision 25
# speedup vs baseline: 1.1656x; 1.1656x over previous
"""BitLinearLRLS fused fp8-DoubleRow kernel for 8 Trainium2 NeuronCores.

Math (see reference):
    w_q       = clip(round(weight / 0.5), -1, 1)            # ternary, RNE ties
    x_mean    = mean(x, axis=(0,1))                         # [in]
    scale_eff = scale + lrls_A @ (lrls_B @ x_mean)          # [out]
    y         = x @ (w_q * scale_eff[:, None]).T

Key identity: y = (x @ w_q.T) * scale_eff[None, :] — the big matmul does not
depend on scale_eff, so the data-dependent scale is a per-output-row epilogue.

Precision: x is cast to fp8 e4m3 on the host (RNE); w_q in {-1,0,1} is exact
in fp8. Measured end-to-end max rel err on the seeded inputs: 1.68e-2
(budget 2e-2). The win: fp8 with perf_mode=DoubleRow packs two contraction
rows per PE cell — a 256-deep k-chunk per 512-column matmul, ~1.5x the
fp32r/bf16 row rate.

Sharding: data-parallel over tokens, tok/8 = 2048 per core. x^T (fp8) is
SBUF-resident (64 KiB/partition), read from HBM exactly once. Weights stream
in 16 slabs of 256 out-features, quantized on the DVE (exact int32-RNE cast
chain), laid out as [128, 2, osize] chunk-pair tiles for DoubleRow.

Engine plan (queues are FIFO — a blocked op stalls everything behind it):
  PE     : 2048 [256x128x512] fp8 DoubleRow matmuls, nothing else
  DVE    : w quantize (mult+max / min+i32cast / i32->fp8 copy), token sums
           (fp8 reduce), LRLS matvec chain (emitted after slab 4's quantize
           so the cc_out wait never starves later quantizes)
  ACT    : epilogues only. Slabs 0..SWITCH-1: unscaled PSUM->SBUF copies
           (frees banks with no scale_eff dependency — covers the AllReduce
           latency), with the scale passes emitted in a burst at the end of
           slab SWITCH-1. Slabs >= SWITCH: direct scaled epilogue.
  sync   : x pair-tile loads, cc_in copy, all y stores
  gpsimd : w quad-chunk loads (SWDGE), collective, LRLS small loads

Quantization is exact vs the reference:
    w_q = fp8(int32_cast_rne(clamp(2w, -1.25, 1.25)))
(the DVE float->int cast is round-to-nearest-even, matching round(); -1/0/1
are exact in fp8 e4m3).
"""

import numpy as np
import ml_dtypes

import concourse.bass as bass
import concourse.tile as tile
from concourse import bacc, mybir
from concourse.bass_utils import run_bass_kernel_spmd

F32 = mybir.dt.float32
F8 = mybir.dt.float8e4
I32 = mybir.dt.int32
ALU = mybir.AluOpType
ACTF = mybir.ActivationFunctionType
DR = mybir.MatmulPerfMode.DoubleRow


class Cfg:
    def __init__(self, tok=16384, din=4096, dout=4096, r=16,
                 tsh=None, oslab=256, tblk=512, ncores=8):
        self.ncores = ncores
        self.tok = tok            # total tokens (B*S)
        self.din = din
        self.dout = dout
        self.r = r
        self.tsh = tsh or tok // ncores   # tokens per core
        self.oslab = oslab        # output features per weight slab
        self.tblk = tblk          # moving-operand tile (tokens)
        self.kc = din // 128      # 128-row contraction chunks
        self.kc2 = din // 256     # 256-row DoubleRow chunk pairs
        self.kc4 = din // 512     # 512-row w-load quads
        self.oc = dout // 128     # output 128-chunks (scale_eff columns)
        self.nslab = dout // oslab
        self.ntblk = self.tsh // tblk
        self.nos = oslab // 128   # psum groups per t-block (out dim)
        # slabs < direct_slab stage unscaled y in DRAM scratch and get fixed
        # up one-per-slab from direct_slab on; robust to ~300us of NEFF
        # launch skew feeding the AllReduce
        self.direct_slab = 8
        self.bchunk = min(512, din // 4)   # LRLS B matvec chunk
        assert self.nos * self.ntblk <= 8
        assert din % self.bchunk == 0


def build(cfg: Cfg, compile=True):
    nc = bacc.Bacc("TRN2", target_bir_lowering=False, debug=False,
                   enable_asserts=True, num_devices=cfg.ncores)

    x8T = nc.dram_tensor("x8T", [cfg.din, cfg.tsh], F8,
                         kind="ExternalInput").ap()
    wT = nc.dram_tensor("wT", [cfg.din, cfg.dout], F32,
                        kind="ExternalInput").ap()
    scale_pc = nc.dram_tensor("scale_pc", [128, cfg.oc], F32,
                              kind="ExternalInput").ap()
    # b_pk[r, p*kc + k] = B[r, k*128 + p]  (matches sums' [p, k] flat order)
    b_pk = nc.dram_tensor("b_pk", [cfg.r, cfg.din], F32,
                          kind="ExternalInput").ap()
    # a_p[p, j*r + rr] = A[j*128 + p, rr]
    a_p = nc.dram_tensor("a_p", [128, cfg.oc * cfg.r], F32,
                         kind="ExternalInput").ap()
    yT = nc.dram_tensor("yT", [cfg.dout, cfg.tsh], F32,
                        kind="ExternalOutput").ap()

    with tile.TileContext(nc) as tc:
        with tc.tile_pool(name="keep", bufs=1) as keep, \
             tc.tile_pool(name="cdram", bufs=1, space="DRAM") as cdram, \
             tc.tile_pool(name="xp", bufs=cfg.kc2) as xpp, \
             tc.tile_pool(name="wst", bufs=4) as wst, \
             tc.tile_pool(name="qa", bufs=2) as qap, \
             tc.tile_pool(name="qb", bufs=2) as qbp, \
             tc.tile_pool(name="wq", bufs=2 * cfg.kc2) as wqp, \
             tc.tile_pool(name="yt", bufs=6) as ytp, \
             tc.tile_pool(name="ft", bufs=6) as ftp, \
             tc.tile_pool(name="ft2", bufs=4) as ft2p, \
             tc.tile_pool(name="lr", bufs=1) as lr, \
             tc.tile_pool(name="lr2", bufs=1) as lr2:

            sums = keep.tile([128, cfg.kc], F32)
            scale_eff = keep.tile([128, cfg.oc], F32)
            trash = keep.tile([128, cfg.tsh], F8)

            cc_in = cdram.tile([128, cfg.kc], F32)
            cc_out = cdram.tile([128, cfg.kc], F32)
            v_d = cdram.tile([1, cfg.r], F32)

            # ---- x: load fp8 x^T once, resident as chunk-pair tiles.
            # Token sums ride ACT (idle until the first epilogue) as
            # Copy-with-accum_out self-reduces, keeping the DVE free for the
            # weight quantize cadence the PE is fed by. ----
            xp = []
            for c in range(cfg.kc2):
                xt = xpp.tile([128, 2, cfg.tsh], F8, name="xt")
                nc.sync.dma_start(
                    out=xt,
                    in_=x8T[c * 256:(c + 1) * 256, :]
                    .rearrange("(two p) t -> p two t", p=128))
                xp.append(xt)
            for ib in range(cfg.kc // 2):
                nc.scalar.activation(
                    out=trash, in_=xp[ib // 2][:, ib % 2, :],
                    func=ACTF.Copy, accum_out=sums[:, ib:ib + 1])

            def emit_lrls():
                # scale_eff = scale + A @ (B @ sum_x) / tok, pure DVE epilogue
                nchunk = cfg.din // cfg.bchunk
                vparts = lr.tile([cfg.r, nchunk], F32)
                for c in range(nchunk):
                    xbc = lr2.tile([cfg.r, cfg.bchunk], F32, name="xbc")
                    nc.gpsimd.dma_start(
                        out=xbc,
                        in_=bass.AP(tensor=cc_out.tensor,
                                    offset=cc_out.offset + c * cfg.bchunk,
                                    ap=[[0, cfg.r], [1, cfg.bchunk]]))
                    bsb = lr2.tile([cfg.r, cfg.bchunk], F32, name="bsb")
                    nc.gpsimd.dma_start(
                        out=bsb,
                        in_=b_pk[:, c * cfg.bchunk:(c + 1) * cfg.bchunk])
                    nc.vector.tensor_tensor(out=xbc, in0=bsb, in1=xbc,
                                            op=ALU.mult)
                    nc.vector.reduce_sum(out=vparts[:, c:c + 1], in_=xbc,
                                         axis=mybir.AxisListType.X)
                vsb = lr.tile([cfg.r, 1], F32)
                nc.vector.reduce_sum(out=vsb, in_=vparts,
                                     axis=mybir.AxisListType.X)
                nc.gpsimd.dma_start(out=v_d.rearrange("one r -> r one"),
                                    in_=vsb)

                # corr[p, j] = sum_rr a_p[p, j*r+rr] * v[rr]
                vb16 = lr.tile([128, cfg.r], F32)
                nc.gpsimd.dma_start(
                    out=vb16,
                    in_=bass.AP(tensor=v_d.tensor, offset=v_d.offset,
                                ap=[[0, 128], [1, cfg.r]]))
                ap_sb = lr.tile([128, cfg.oc, cfg.r], F32)
                nc.gpsimd.dma_start(
                    out=ap_sb,
                    in_=a_p.rearrange("p (j rr) -> p j rr", rr=cfg.r))
                am = lr.tile([128, cfg.oc, cfg.r], F32)
                for j in range(cfg.oc):
                    nc.vector.tensor_tensor(out=am[:, j, :],
                                            in0=ap_sb[:, j, :],
                                            in1=vb16, op=ALU.mult)
                corr = lr.tile([128, cfg.oc], F32)
                nc.vector.reduce_sum(out=corr, in_=am,
                                     axis=mybir.AxisListType.X)
                sc_sb = lr.tile([128, cfg.oc], F32)
                nc.gpsimd.dma_start(out=sc_sb, in_=scale_pc)
                nc.vector.tensor_scalar(out=scale_eff, in0=corr,
                                        scalar1=1.0 / cfg.tok, scalar2=None,
                                        op0=ALU.mult)
                nc.vector.tensor_tensor(out=scale_eff, in0=scale_eff,
                                        in1=sc_sb, op=ALU.add)

            deferred = []   # (stage_tile, oi, tb) awaiting scale_eff

            with tc.tile_pool(name="ps", bufs=8, space="PSUM") as psp:
                for s in range(cfg.nslab):
                    ostart = s * cfg.oslab

                    # ---- w slab load (512-row quads) + quantize (pairs) ----
                    wq_tiles = []
                    for kb in range(cfg.kc4):
                        wr = wst.tile([128, 4, cfg.oslab], F32, name="wr")
                        nc.gpsimd.dma_start(
                            out=wr,
                            in_=wT[kb * 512:(kb + 1) * 512,
                                   ostart:ostart + cfg.oslab]
                            .rearrange("(four p) o -> p four o", p=128))
                        ga = qap.tile([128, 4, cfg.oslab], F32, name="ga")
                        nc.vector.tensor_scalar(
                            out=ga, in0=wr, scalar1=2.0, scalar2=-1.25,
                            op0=ALU.mult, op1=ALU.max)
                        gb = qbp.tile([128, 4, cfg.oslab], I32, name="gb")
                        nc.vector.tensor_scalar(
                            out=gb, in0=ga, scalar1=1.25, scalar2=None,
                            op0=ALU.min)
                        for half in range(2):
                            wq_t = wqp.tile([128, 2, cfg.oslab], F8,
                                            name="wq_t")
                            nc.vector.tensor_copy(
                                out=wq_t, in_=gb[:, 2 * half:2 * half + 2, :])
                            wq_tiles.append(wq_t)
                        if s == 1:
                            # second half of the token sums, woven through
                            # the DVE stream once all x pairs have landed
                            for i in range(2):
                                ib = cfg.kc // 2 + 2 * kb + i
                                nc.vector.reduce_sum(
                                    out=sums[:, ib:ib + 1],
                                    in_=xp[ib // 2][:, ib % 2, :],
                                    axis=mybir.AxisListType.X)
                    if s == 1:
                        nc.sync.dma_start(out=cc_in, in_=sums)
                        nc.gpsimd.collective_compute(
                            "AllReduce", ALU.add,
                            replica_groups=[list(range(cfg.ncores))],
                            ins=[cc_in.opt()], outs=[cc_out.opt()],
                        )
                    if s == cfg.direct_slab:
                        # Tile deps are program-order: scale_eff's producers
                        # must be emitted before any consumer. By the time
                        # the DVE reaches these ops (wq-pool gated, ~2 slabs
                        # behind s) cc_out has landed even at worst-case
                        # launch skew, so the wait never starves quantize.
                        emit_lrls()

                    # ---- matmuls: 256-deep DoubleRow chunks ----
                    pst = [[psp.tile([128, cfg.tblk], F32, name="pst")
                            for _ in range(cfg.ntblk)]
                           for _ in range(cfg.nos)]
                    for c in range(cfg.kc2):
                        for os_ in range(cfg.nos):
                            lhs = wq_tiles[c][:, :,
                                              os_ * 128:(os_ + 1) * 128]
                            for tb in range(cfg.ntblk):
                                nc.tensor.matmul(
                                    pst[os_][tb],
                                    lhsT=lhs,
                                    rhs=xp[c][:, :,
                                              tb * cfg.tblk:(tb + 1) * cfg.tblk],
                                    start=(c == 0), stop=(c == cfg.kc2 - 1),
                                    perf_mode=DR,
                                )

                    # ---- epilogues ----
                    # Slabs < direct_slab: scale_eff may not exist yet
                    # (launch skew puts cc_out anywhere in 130..310 us), so
                    # write UNSCALED y to DRAM scratch — the bank-freeing
                    # path has no scale dependency at all. Each such slab is
                    # fixed up (read back, scale, store) direct_slab slabs
                    # later, fully overlapped under remaining PE work.
                    for os_ in range(cfg.nos):
                        oi = ostart // 128 + os_
                        for tb in range(cfg.ntblk):
                            ytt = ytp.tile([128, cfg.tblk], F32, name="ytt")
                            if s < cfg.direct_slab:
                                nc.scalar.activation(
                                    out=ytt, in_=pst[os_][tb],
                                    func=ACTF.Copy)
                                sg = cdram.tile([128, cfg.tblk], F32,
                                                name="scr")
                                nc.scalar.dma_start(out=sg, in_=ytt)
                                deferred.append((sg, oi, tb))
                            else:
                                nc.scalar.activation(
                                    out=ytt, in_=pst[os_][tb],
                                    func=ACTF.Copy,
                                    scale=scale_eff[:, oi:oi + 1])
                                nc.scalar.dma_start(
                                    out=yT[oi * 128:(oi + 1) * 128,
                                           tb * cfg.tblk:(tb + 1) * cfg.tblk],
                                    in_=ytt)

                    if s >= cfg.direct_slab:
                        # fix up one staged slab: scratch -> scale -> yT
                        for sg, oi, tb in deferred[(s - cfg.direct_slab) * 8:
                                                   (s - cfg.direct_slab + 1) * 8]:
                            ft = ftp.tile([128, cfg.tblk], F32, name="ft")
                            nc.sync.dma_start(out=ft, in_=sg)
                            ft2 = ft2p.tile([128, cfg.tblk], F32, name="ft2")
                            nc.scalar.activation(
                                out=ft2, in_=ft, func=ACTF.Copy,
                                scale=scale_eff[:, oi:oi + 1])
                            nc.scalar.dma_start(
                                out=yT[oi * 128:(oi + 1) * 128,
                                       tb * cfg.tblk:(tb + 1) * cfg.tblk],
                                in_=ft2)

    if compile:
        nc.compile()
    return nc


def prep_inputs(cfg: Cfg, x, weight, scale, lrls_A, lrls_B):
    """Host-side sharding/layout marshalling + the fp8 input cast."""
    x_flat = np.ascontiguousarray(x.reshape(cfg.tok, cfg.din))
    x8 = x_flat.astype(ml_dtypes.float8_e4m3)         # RNE
    x8T_full = np.ascontiguousarray(x8.T)             # [din, tok] fp8
    wT = np.ascontiguousarray(weight.T)               # [din, dout]
    b_pk = np.ascontiguousarray(
        lrls_B.reshape(cfg.r, cfg.kc, 128).transpose(0, 2, 1).reshape(
            cfg.r, cfg.din))
    a_p = np.ascontiguousarray(
        lrls_A.reshape(cfg.oc, 128, cfg.r).transpose(1, 0, 2).reshape(
            128, cfg.oc * cfg.r))
    scale_pc = np.ascontiguousarray(scale.reshape(cfg.oc, 128).T)

    in_maps = []
    for c in range(cfg.ncores):
        x8T_c = np.ascontiguousarray(
            x8T_full[:, c * cfg.tsh:(c + 1) * cfg.tsh])
        in_maps.append({"x8T": x8T_c, "wT": wT, "scale_pc": scale_pc,
                        "b_pk": b_pk, "a_p": a_p})
    return in_maps


def assemble_output(cfg: Cfg, results, out_shape):
    y_flat = np.empty((cfg.tok, cfg.dout), np.float32)
    for c in range(cfg.ncores):
        y_flat[c * cfg.tsh:(c + 1) * cfg.tsh, :] = results[c]["yT"].T
    return y_flat.reshape(out_shape)


_NC_CACHE = {}


def run(cfg: Cfg, x, weight, scale, lrls_A, lrls_B, out_shape, **run_kwargs):
    key = (cfg.tok, cfg.din, cfg.dout, cfg.tsh, cfg.oslab, cfg.tblk,
           cfg.direct_slab)
    if key not in _NC_CACHE:
        _NC_CACHE[key] = build(cfg)
    nc = _NC_CACHE[key]
    in_maps = prep_inputs(cfg, x, weight, scale, lrls_A, lrls_B)
    res = run_bass_kernel_spmd(nc, in_maps, core_ids=list(range(cfg.ncores)),
                               **run_kwargs)
    y = assemble_output(cfg, res.results, out_shape)
    return y, res


def kernel(x, weight, threshold, scale, lrls_A, lrls_B):
    # threshold input is unused: the reference hardcodes THRESH=0.5
    # (TrainState.threshold() at step 0), so the ternary cut sits at |w|=0.25.
    cfg = Cfg()
    x = np.asarray(x, np.float32)
    y, _ = run(cfg, x, np.asarray(weight, np.float32),
               np.asarray(scale, np.float32), np.asarray(lrls_A, np.float32),
               np.asarray(lrls_B, np.float32),
               out_shape=(x.shape[0], x.shape[1], np.asarray(weight).shape[0]))
    return y.astype(np.float32)


# revision 28
# speedup vs baseline: 1.2334x; 1.0581x over previous
"""BitLinearLRLS fused fp8-DoubleRow kernel for 8 Trainium2 NeuronCores.

Math (see reference):
    w_q       = clip(round(weight / 0.5), -1, 1)            # ternary, RNE ties
    x_mean    = mean(x, axis=(0,1))                         # [in]
    scale_eff = scale + lrls_A @ (lrls_B @ x_mean)          # [out]
    y         = x @ (w_q * scale_eff[:, None]).T

Key identity: y = (x @ w_q.T) * scale_eff[None, :] — the big matmul does not
depend on scale_eff, so the data-dependent scale is a per-output-row epilogue.

Precision: x is cast to fp8 e4m3 on the host (RNE); w_q in {-1,0,1} is exact
in fp8. Measured end-to-end max rel err on the seeded inputs: 1.68e-2
(budget 2e-2). The win: fp8 with perf_mode=DoubleRow packs two contraction
rows per PE cell — a 256-deep k-chunk per 512-column matmul, ~1.5x the
fp32r/bf16 row rate.

Sharding: data-parallel over tokens, tok/8 = 2048 per core. x^T (fp8) is
SBUF-resident (64 KiB/partition), read from HBM exactly once. Weights stream
in 16 slabs of 256 out-features, quantized on the DVE (exact int32-RNE cast
chain), laid out as [128, 2, osize] chunk-pair tiles for DoubleRow.

Engine plan (queues are FIFO — a blocked op stalls everything behind it):
  PE     : 2048 [256x128x512] fp8 DoubleRow matmuls, nothing else
  DVE    : w quantize (mult+max / min+i32cast / i32->fp8 copy), token sums
           (fp8 reduce), LRLS matvec chain (emitted after slab 4's quantize
           so the cc_out wait never starves later quantizes)
  ACT    : epilogues only. Slabs 0..SWITCH-1: unscaled PSUM->SBUF copies
           (frees banks with no scale_eff dependency — covers the AllReduce
           latency), with the scale passes emitted in a burst at the end of
           slab SWITCH-1. Slabs >= SWITCH: direct scaled epilogue.
  sync   : x pair-tile loads, cc_in copy, all y stores
  gpsimd : w quad-chunk loads (SWDGE), collective, LRLS small loads

Quantization is exact vs the reference:
    w_q = fp8(int32_cast_rne(clamp(2w, -1.25, 1.25)))
(the DVE float->int cast is round-to-nearest-even, matching round(); -1/0/1
are exact in fp8 e4m3).
"""

import numpy as np
import ml_dtypes

import concourse.bass as bass
import concourse.tile as tile
from concourse import bacc, mybir
from concourse.bass_utils import run_bass_kernel_spmd

F32 = mybir.dt.float32
F8 = mybir.dt.float8e4
I32 = mybir.dt.int32
ALU = mybir.AluOpType
ACTF = mybir.ActivationFunctionType
DR = mybir.MatmulPerfMode.DoubleRow


class Cfg:
    def __init__(self, tok=16384, din=4096, dout=4096, r=16,
                 tsh=None, oslab=256, tblk=512, ncores=8):
        self.ncores = ncores
        self.tok = tok            # total tokens (B*S)
        self.din = din
        self.dout = dout
        self.r = r
        self.tsh = tsh or tok // ncores   # tokens per core
        self.oslab = oslab        # output features per weight slab
        self.tblk = tblk          # moving-operand tile (tokens)
        self.kc = din // 128      # 128-row contraction chunks
        self.kc2 = din // 256     # 256-row DoubleRow chunk pairs
        self.kc4 = din // 512     # 512-row w-load quads
        self.oc = dout // 128     # output 128-chunks (scale_eff columns)
        self.nslab = dout // oslab
        self.ntblk = self.tsh // tblk
        self.nos = oslab // 128   # psum groups per t-block (out dim)
        # slabs < direct_slab stage unscaled y in DRAM scratch and get fixed
        # up one-per-slab from direct_slab on; robust to ~300us of NEFF
        # launch skew feeding the AllReduce
        self.direct_slab = 8
        self.bchunk = min(512, din // 4)   # LRLS B matvec chunk
        assert self.nos * self.ntblk <= 8
        assert din % self.bchunk == 0


def build(cfg: Cfg, compile=True):
    nc = bacc.Bacc("TRN2", target_bir_lowering=False, debug=False,
                   enable_asserts=True, num_devices=cfg.ncores)

    x8T = nc.dram_tensor("x8T", [cfg.din, cfg.tsh], F8,
                         kind="ExternalInput").ap()
    wT = nc.dram_tensor("wT", [cfg.din, cfg.dout], F32,
                        kind="ExternalInput").ap()
    scale_pc = nc.dram_tensor("scale_pc", [128, cfg.oc], F32,
                              kind="ExternalInput").ap()
    # b_pk[r, p*kc + k] = B[r, k*128 + p]  (matches sums' [p, k] flat order)
    b_pk = nc.dram_tensor("b_pk", [cfg.r, cfg.din], F32,
                          kind="ExternalInput").ap()
    # a_p[p, j*r + rr] = A[j*128 + p, rr]
    a_p = nc.dram_tensor("a_p", [128, cfg.oc * cfg.r], F32,
                         kind="ExternalInput").ap()
    yT = nc.dram_tensor("yT", [cfg.dout, cfg.tsh], F32,
                        kind="ExternalOutput").ap()

    with tile.TileContext(nc) as tc:
        with tc.tile_pool(name="keep", bufs=1) as keep, \
             tc.tile_pool(name="cdram", bufs=1, space="DRAM") as cdram, \
             tc.tile_pool(name="xp", bufs=cfg.kc2) as xpp, \
             tc.tile_pool(name="wst", bufs=4) as wst, \
             tc.tile_pool(name="qa", bufs=2) as qap, \
             tc.tile_pool(name="qb", bufs=2) as qbp, \
             tc.tile_pool(name="wq", bufs=3 * cfg.kc2) as wqp, \
             tc.tile_pool(name="yt", bufs=6) as ytp, \
             tc.tile_pool(name="ft", bufs=6) as ftp, \
             tc.tile_pool(name="ft2", bufs=4) as ft2p, \
             tc.tile_pool(name="lr", bufs=1) as lr, \
             tc.tile_pool(name="lr2", bufs=1) as lr2:

            sums = keep.tile([128, cfg.kc], F32)
            scale_eff = keep.tile([128, cfg.oc], F32)
            trash = keep.tile([128, cfg.tsh], F8)

            cc_in = cdram.tile([128, cfg.kc], F32)
            cc_out = cdram.tile([128, cfg.kc], F32)
            v_d = cdram.tile([1, cfg.r], F32)

            # ---- x: load fp8 x^T once, resident as chunk-pair tiles.
            # Token sums ride ACT (idle until the first epilogue) as
            # Copy-with-accum_out self-reduces, keeping the DVE free for the
            # weight quantize cadence the PE is fed by. ----
            xp = []
            for c in range(cfg.kc2):
                xt = xpp.tile([128, 2, cfg.tsh], F8, name="xt")
                nc.sync.dma_start(
                    out=xt,
                    in_=x8T[c * 256:(c + 1) * 256, :]
                    .rearrange("(two p) t -> p two t", p=128))
                xp.append(xt)
            for ib in range(cfg.kc // 2):
                nc.scalar.activation(
                    out=trash, in_=xp[ib // 2][:, ib % 2, :],
                    func=ACTF.Copy, accum_out=sums[:, ib:ib + 1])

            def emit_lrls():
                # scale_eff = scale + A @ (B @ sum_x) / tok, pure DVE epilogue
                nchunk = cfg.din // cfg.bchunk
                vparts = lr.tile([cfg.r, nchunk], F32)
                for c in range(nchunk):
                    xbc = lr2.tile([cfg.r, cfg.bchunk], F32, name="xbc")
                    nc.gpsimd.dma_start(
                        out=xbc,
                        in_=bass.AP(tensor=cc_out.tensor,
                                    offset=cc_out.offset + c * cfg.bchunk,
                                    ap=[[0, cfg.r], [1, cfg.bchunk]]))
                    bsb = lr2.tile([cfg.r, cfg.bchunk], F32, name="bsb")
                    nc.gpsimd.dma_start(
                        out=bsb,
                        in_=b_pk[:, c * cfg.bchunk:(c + 1) * cfg.bchunk])
                    nc.vector.tensor_tensor(out=xbc, in0=bsb, in1=xbc,
                                            op=ALU.mult)
                    nc.vector.reduce_sum(out=vparts[:, c:c + 1], in_=xbc,
                                         axis=mybir.AxisListType.X)
                vsb = lr.tile([cfg.r, 1], F32)
                nc.vector.reduce_sum(out=vsb, in_=vparts,
                                     axis=mybir.AxisListType.X)
                nc.gpsimd.dma_start(out=v_d.rearrange("one r -> r one"),
                                    in_=vsb)

                # corr[p, j] = sum_rr a_p[p, j*r+rr] * v[rr]
                vb16 = lr.tile([128, cfg.r], F32)
                nc.gpsimd.dma_start(
                    out=vb16,
                    in_=bass.AP(tensor=v_d.tensor, offset=v_d.offset,
                                ap=[[0, 128], [1, cfg.r]]))
                ap_sb = lr.tile([128, cfg.oc, cfg.r], F32)
                nc.gpsimd.dma_start(
                    out=ap_sb,
                    in_=a_p.rearrange("p (j rr) -> p j rr", rr=cfg.r))
                am = lr.tile([128, cfg.oc, cfg.r], F32)
                for j in range(cfg.oc):
                    nc.vector.tensor_tensor(out=am[:, j, :],
                                            in0=ap_sb[:, j, :],
                                            in1=vb16, op=ALU.mult)
                corr = lr.tile([128, cfg.oc], F32)
                nc.vector.reduce_sum(out=corr, in_=am,
                                     axis=mybir.AxisListType.X)
                sc_sb = lr.tile([128, cfg.oc], F32)
                nc.gpsimd.dma_start(out=sc_sb, in_=scale_pc)
                nc.vector.tensor_scalar(out=scale_eff, in0=corr,
                                        scalar1=1.0 / cfg.tok, scalar2=None,
                                        op0=ALU.mult)
                nc.vector.tensor_tensor(out=scale_eff, in0=scale_eff,
                                        in1=sc_sb, op=ALU.add)

            deferred = []   # (stage_tile, oi, tb) awaiting scale_eff

            with tc.tile_pool(name="ps", bufs=8, space="PSUM") as psp:
                for s in range(cfg.nslab):
                    ostart = s * cfg.oslab

                    # ---- w slab load (512-row quads) + quantize (pairs) ----
                    wq_tiles = []
                    for kb in range(cfg.kc4):
                        wr = wst.tile([128, 4, cfg.oslab], F32, name="wr")
                        nc.gpsimd.dma_start(
                            out=wr,
                            in_=wT[kb * 512:(kb + 1) * 512,
                                   ostart:ostart + cfg.oslab]
                            .rearrange("(four p) o -> p four o", p=128))
                        ga = qap.tile([128, 4, cfg.oslab], F32, name="ga")
                        nc.vector.tensor_scalar(
                            out=ga, in0=wr, scalar1=2.0, scalar2=-1.25,
                            op0=ALU.mult, op1=ALU.max)
                        gb = qbp.tile([128, 4, cfg.oslab], I32, name="gb")
                        nc.vector.tensor_scalar(
                            out=gb, in0=ga, scalar1=1.25, scalar2=None,
                            op0=ALU.min)
                        for half in range(2):
                            wq_t = wqp.tile([128, 2, cfg.oslab], F8,
                                            name="wq_t")
                            nc.vector.tensor_copy(
                                out=wq_t, in_=gb[:, 2 * half:2 * half + 2, :])
                            wq_tiles.append(wq_t)
                        if s == 1:
                            # second half of the token sums, woven through
                            # the DVE stream once all x pairs have landed
                            for i in range(2):
                                ib = cfg.kc // 2 + 2 * kb + i
                                nc.vector.reduce_sum(
                                    out=sums[:, ib:ib + 1],
                                    in_=xp[ib // 2][:, ib % 2, :],
                                    axis=mybir.AxisListType.X)
                    if s == 1:
                        nc.sync.dma_start(out=cc_in, in_=sums)
                        nc.gpsimd.collective_compute(
                            "AllReduce", ALU.add,
                            replica_groups=[list(range(cfg.ncores))],
                            ins=[cc_in.opt()], outs=[cc_out.opt()],
                        )
                    if s == cfg.direct_slab:
                        # Tile deps are program-order: scale_eff's producers
                        # must be emitted before any consumer. By the time
                        # the DVE reaches these ops (wq-pool gated, ~3 slabs
                        # behind s) cc_out has landed even at worst-case
                        # launch skew, so the wait never starves quantize.
                        emit_lrls()
                    if s >= cfg.direct_slab:
                        # fix up one staged slab (scratch -> scale -> yT),
                        # emitted BEFORE this slab's own epilogues so the
                        # last fixup drains under the final slab's matmuls
                        # instead of serializing after them
                        for sg, oi, tb in deferred[(s - cfg.direct_slab) * 8:
                                                   (s - cfg.direct_slab + 1) * 8]:
                            ft = ftp.tile([128, cfg.tblk], F32, name="ft")
                            nc.sync.dma_start(out=ft, in_=sg)
                            ft2 = ft2p.tile([128, cfg.tblk], F32, name="ft2")
                            nc.scalar.activation(
                                out=ft2, in_=ft, func=ACTF.Copy,
                                scale=scale_eff[:, oi:oi + 1])
                            nc.scalar.dma_start(
                                out=yT[oi * 128:(oi + 1) * 128,
                                       tb * cfg.tblk:(tb + 1) * cfg.tblk],
                                in_=ft2)

                    # ---- matmuls: 256-deep DoubleRow chunks ----
                    pst = [[psp.tile([128, cfg.tblk], F32, name="pst")
                            for _ in range(cfg.ntblk)]
                           for _ in range(cfg.nos)]
                    for c in range(cfg.kc2):
                        for os_ in range(cfg.nos):
                            lhs = wq_tiles[c][:, :,
                                              os_ * 128:(os_ + 1) * 128]
                            for tb in range(cfg.ntblk):
                                nc.tensor.matmul(
                                    pst[os_][tb],
                                    lhsT=lhs,
                                    rhs=xp[c][:, :,
                                              tb * cfg.tblk:(tb + 1) * cfg.tblk],
                                    start=(c == 0), stop=(c == cfg.kc2 - 1),
                                    perf_mode=DR,
                                )

                    # ---- epilogues ----
                    # Slabs < direct_slab: scale_eff may not exist yet
                    # (launch skew puts cc_out anywhere in 130..310 us), so
                    # write UNSCALED y to DRAM scratch — the bank-freeing
                    # path has no scale dependency at all. Each such slab is
                    # fixed up (read back, scale, store) direct_slab slabs
                    # later, fully overlapped under remaining PE work.
                    for os_ in range(cfg.nos):
                        oi = ostart // 128 + os_
                        for tb in range(cfg.ntblk):
                            ytt = ytp.tile([128, cfg.tblk], F32, name="ytt")
                            if s < cfg.direct_slab:
                                nc.scalar.activation(
                                    out=ytt, in_=pst[os_][tb],
                                    func=ACTF.Copy)
                                sg = cdram.tile([128, cfg.tblk], F32,
                                                name="scr")
                                nc.scalar.dma_start(out=sg, in_=ytt)
                                deferred.append((sg, oi, tb))
                            else:
                                nc.scalar.activation(
                                    out=ytt, in_=pst[os_][tb],
                                    func=ACTF.Copy,
                                    scale=scale_eff[:, oi:oi + 1])
                                nc.scalar.dma_start(
                                    out=yT[oi * 128:(oi + 1) * 128,
                                           tb * cfg.tblk:(tb + 1) * cfg.tblk],
                                    in_=ytt)



    if compile:
        nc.compile()
    return nc


def prep_inputs(cfg: Cfg, x, weight, scale, lrls_A, lrls_B):
    """Host-side sharding/layout marshalling + the fp8 input cast."""
    x_flat = np.ascontiguousarray(x.reshape(cfg.tok, cfg.din))
    x8 = x_flat.astype(ml_dtypes.float8_e4m3)         # RNE
    x8T_full = np.ascontiguousarray(x8.T)             # [din, tok] fp8
    wT = np.ascontiguousarray(weight.T)               # [din, dout]
    b_pk = np.ascontiguousarray(
        lrls_B.reshape(cfg.r, cfg.kc, 128).transpose(0, 2, 1).reshape(
            cfg.r, cfg.din))
    a_p = np.ascontiguousarray(
        lrls_A.reshape(cfg.oc, 128, cfg.r).transpose(1, 0, 2).reshape(
            128, cfg.oc * cfg.r))
    scale_pc = np.ascontiguousarray(scale.reshape(cfg.oc, 128).T)

    in_maps = []
    for c in range(cfg.ncores):
        x8T_c = np.ascontiguousarray(
            x8T_full[:, c * cfg.tsh:(c + 1) * cfg.tsh])
        in_maps.append({"x8T": x8T_c, "wT": wT, "scale_pc": scale_pc,
                        "b_pk": b_pk, "a_p": a_p})
    return in_maps


def assemble_output(cfg: Cfg, results, out_shape):
    y_flat = np.empty((cfg.tok, cfg.dout), np.float32)
    for c in range(cfg.ncores):
        y_flat[c * cfg.tsh:(c + 1) * cfg.tsh, :] = results[c]["yT"].T
    return y_flat.reshape(out_shape)


_NC_CACHE = {}


def run(cfg: Cfg, x, weight, scale, lrls_A, lrls_B, out_shape, **run_kwargs):
    key = (cfg.tok, cfg.din, cfg.dout, cfg.tsh, cfg.oslab, cfg.tblk,
           cfg.direct_slab)
    if key not in _NC_CACHE:
        _NC_CACHE[key] = build(cfg)
    nc = _NC_CACHE[key]
    in_maps = prep_inputs(cfg, x, weight, scale, lrls_A, lrls_B)
    res = run_bass_kernel_spmd(nc, in_maps, core_ids=list(range(cfg.ncores)),
                               **run_kwargs)
    y = assemble_output(cfg, res.results, out_shape)
    return y, res


def kernel(x, weight, threshold, scale, lrls_A, lrls_B):
    # threshold input is unused: the reference hardcodes THRESH=0.5
    # (TrainState.threshold() at step 0), so the ternary cut sits at |w|=0.25.
    cfg = Cfg()
    x = np.asarray(x, np.float32)
    y, _ = run(cfg, x, np.asarray(weight, np.float32),
               np.asarray(scale, np.float32), np.asarray(lrls_A, np.float32),
               np.asarray(lrls_B, np.float32),
               out_shape=(x.shape[0], x.shape[1], np.asarray(weight).shape[0]))
    return y.astype(np.float32)


# revision 31
# speedup vs baseline: 1.2393x; 1.0049x over previous
"""BitLinearLRLS fused fp8-DoubleRow kernel for 8 Trainium2 NeuronCores.

Math (see reference):
    w_q       = clip(round(weight / 0.5), -1, 1)            # ternary, RNE ties
    x_mean    = mean(x, axis=(0,1))                         # [in]
    scale_eff = scale + lrls_A @ (lrls_B @ x_mean)          # [out]
    y         = x @ (w_q * scale_eff[:, None]).T

Key identity: y = (x @ w_q.T) * scale_eff[None, :] — the big matmul does not
depend on scale_eff, so the data-dependent scale is a per-output-row epilogue.

Precision: x is cast to fp8 e4m3 on the host (RNE); w_q in {-1,0,1} is exact
in fp8. Measured end-to-end max rel err on the seeded inputs: 1.68e-2
(budget 2e-2). The win: fp8 with perf_mode=DoubleRow packs two contraction
rows per PE cell — a 256-deep k-chunk per 512-column matmul, ~1.5x the
fp32r/bf16 row rate.

Sharding: data-parallel over tokens, tok/8 = 2048 per core. x^T (fp8) is
SBUF-resident (64 KiB/partition), read from HBM exactly once. Weights stream
in 16 slabs of 256 out-features, quantized on the DVE (exact int32-RNE cast
chain), laid out as [128, 2, osize] chunk-pair tiles for DoubleRow.

Engine plan (queues are FIFO — a blocked op stalls everything behind it):
  PE     : 2048 [256x128x512] fp8 DoubleRow matmuls, nothing else
  DVE    : w quantize (mult+max / min+i32cast / i32->fp8 copy), token sums
           (fp8 reduce), LRLS matvec chain (emitted after slab 4's quantize
           so the cc_out wait never starves later quantizes)
  ACT    : epilogues only. Slabs 0..SWITCH-1: unscaled PSUM->SBUF copies
           (frees banks with no scale_eff dependency — covers the AllReduce
           latency), with the scale passes emitted in a burst at the end of
           slab SWITCH-1. Slabs >= SWITCH: direct scaled epilogue.
  sync   : x pair-tile loads, cc_in copy, all y stores
  gpsimd : w quad-chunk loads (SWDGE), collective, LRLS small loads

Quantization is exact vs the reference:
    w_q = fp8(int32_cast_rne(clamp(2w, -1.25, 1.25)))
(the DVE float->int cast is round-to-nearest-even, matching round(); -1/0/1
are exact in fp8 e4m3).
"""

import numpy as np
import ml_dtypes

import concourse.bass as bass
import concourse.tile as tile
from concourse import bacc, mybir
from concourse.bass_utils import run_bass_kernel_spmd

F32 = mybir.dt.float32
F8 = mybir.dt.float8e4
I32 = mybir.dt.int32
ALU = mybir.AluOpType
ACTF = mybir.ActivationFunctionType
DR = mybir.MatmulPerfMode.DoubleRow


class Cfg:
    def __init__(self, tok=16384, din=4096, dout=4096, r=16,
                 tsh=None, oslab=256, tblk=512, ncores=8):
        self.ncores = ncores
        self.tok = tok            # total tokens (B*S)
        self.din = din
        self.dout = dout
        self.r = r
        self.tsh = tsh or tok // ncores   # tokens per core
        self.oslab = oslab        # output features per weight slab
        self.tblk = tblk          # moving-operand tile (tokens)
        self.kc = din // 128      # 128-row contraction chunks
        self.kc2 = din // 256     # 256-row DoubleRow chunk pairs
        self.kc4 = din // 512     # 512-row w-load quads
        self.oc = dout // 128     # output 128-chunks (scale_eff columns)
        self.nslab = dout // oslab
        self.ntblk = self.tsh // tblk
        self.nos = oslab // 128   # psum groups per t-block (out dim)
        # slabs < direct_slab stage unscaled y in DRAM scratch and get fixed
        # up one-per-slab from direct_slab on; robust to ~300us of NEFF
        # launch skew feeding the AllReduce
        self.direct_slab = 8
        self.bchunk = min(512, din // 4)   # LRLS B matvec chunk
        assert self.nos * self.ntblk <= 8
        assert din % self.bchunk == 0


def build(cfg: Cfg, compile=True):
    nc = bacc.Bacc("TRN2", target_bir_lowering=False, debug=False,
                   enable_asserts=True, num_devices=cfg.ncores)

    x8T = nc.dram_tensor("x8T", [cfg.din, cfg.tsh], F8,
                         kind="ExternalInput").ap()
    wT = nc.dram_tensor("wT", [cfg.din, cfg.dout], F32,
                        kind="ExternalInput").ap()
    scale_pc = nc.dram_tensor("scale_pc", [128, cfg.oc], F32,
                              kind="ExternalInput").ap()
    # b_pk[r, p*kc + k] = B[r, k*128 + p]  (matches sums' [p, k] flat order)
    b_pk = nc.dram_tensor("b_pk", [cfg.r, cfg.din], F32,
                          kind="ExternalInput").ap()
    # a_p[p, j*r + rr] = A[j*128 + p, rr]
    a_p = nc.dram_tensor("a_p", [128, cfg.oc * cfg.r], F32,
                         kind="ExternalInput").ap()
    yT = nc.dram_tensor("yT", [cfg.dout, cfg.tsh], F32,
                        kind="ExternalOutput").ap()

    with tile.TileContext(nc) as tc:
        with tc.tile_pool(name="keep", bufs=1) as keep, \
             tc.tile_pool(name="cdram", bufs=1, space="DRAM") as cdram, \
             tc.tile_pool(name="xp", bufs=cfg.kc2) as xpp, \
             tc.tile_pool(name="wst", bufs=4) as wst, \
             tc.tile_pool(name="qa", bufs=2) as qap, \
             tc.tile_pool(name="qb", bufs=2) as qbp, \
             tc.tile_pool(name="wq", bufs=3 * cfg.kc2) as wqp, \
             tc.tile_pool(name="yt", bufs=6) as ytp, \
             tc.tile_pool(name="ft", bufs=6) as ftp, \
             tc.tile_pool(name="lr", bufs=1) as lr, \
             tc.tile_pool(name="lr2", bufs=1) as lr2:

            sums = keep.tile([128, cfg.kc], F32)
            scale_eff = keep.tile([128, cfg.oc], F32)
            trash = keep.tile([128, cfg.tsh], F8)

            cc_in = cdram.tile([128, cfg.kc], F32)
            cc_out = cdram.tile([128, cfg.kc], F32)
            v_d = cdram.tile([1, cfg.r], F32)

            # ---- x: load fp8 x^T once, resident as chunk-pair tiles.
            # Token sums ride ACT (idle until the first epilogue) as
            # Copy-with-accum_out self-reduces, keeping the DVE free for the
            # weight quantize cadence the PE is fed by. ----
            xp = []
            for c in range(cfg.kc2):
                xt = xpp.tile([128, 2, cfg.tsh], F8, name="xt")
                nc.sync.dma_start(
                    out=xt,
                    in_=x8T[c * 256:(c + 1) * 256, :]
                    .rearrange("(two p) t -> p two t", p=128))
                xp.append(xt)
            for ib in range(cfg.kc // 2):
                nc.scalar.activation(
                    out=trash, in_=xp[ib // 2][:, ib % 2, :],
                    func=ACTF.Copy, accum_out=sums[:, ib:ib + 1])

            def emit_lrls():
                # scale_eff = scale + A @ (B @ sum_x) / tok, pure DVE epilogue
                nchunk = cfg.din // cfg.bchunk
                vparts = lr.tile([cfg.r, nchunk], F32)
                for c in range(nchunk):
                    xbc = lr2.tile([cfg.r, cfg.bchunk], F32, name="xbc")
                    nc.gpsimd.dma_start(
                        out=xbc,
                        in_=bass.AP(tensor=cc_out.tensor,
                                    offset=cc_out.offset + c * cfg.bchunk,
                                    ap=[[0, cfg.r], [1, cfg.bchunk]]))
                    bsb = lr2.tile([cfg.r, cfg.bchunk], F32, name="bsb")
                    nc.gpsimd.dma_start(
                        out=bsb,
                        in_=b_pk[:, c * cfg.bchunk:(c + 1) * cfg.bchunk])
                    nc.vector.tensor_tensor(out=xbc, in0=bsb, in1=xbc,
                                            op=ALU.mult)
                    nc.vector.reduce_sum(out=vparts[:, c:c + 1], in_=xbc,
                                         axis=mybir.AxisListType.X)
                vsb = lr.tile([cfg.r, 1], F32)
                nc.vector.reduce_sum(out=vsb, in_=vparts,
                                     axis=mybir.AxisListType.X)
                nc.gpsimd.dma_start(out=v_d.rearrange("one r -> r one"),
                                    in_=vsb)

                # corr[p, j] = sum_rr a_p[p, j*r+rr] * v[rr]
                vb16 = lr.tile([128, cfg.r], F32)
                nc.gpsimd.dma_start(
                    out=vb16,
                    in_=bass.AP(tensor=v_d.tensor, offset=v_d.offset,
                                ap=[[0, 128], [1, cfg.r]]))
                ap_sb = lr.tile([128, cfg.oc, cfg.r], F32)
                nc.gpsimd.dma_start(
                    out=ap_sb,
                    in_=a_p.rearrange("p (j rr) -> p j rr", rr=cfg.r))
                am = lr.tile([128, cfg.oc, cfg.r], F32)
                for j in range(cfg.oc):
                    nc.vector.tensor_tensor(out=am[:, j, :],
                                            in0=ap_sb[:, j, :],
                                            in1=vb16, op=ALU.mult)
                corr = lr.tile([128, cfg.oc], F32)
                nc.vector.reduce_sum(out=corr, in_=am,
                                     axis=mybir.AxisListType.X)
                sc_sb = lr.tile([128, cfg.oc], F32)
                nc.gpsimd.dma_start(out=sc_sb, in_=scale_pc)
                nc.vector.tensor_scalar(out=scale_eff, in0=corr,
                                        scalar1=1.0 / cfg.tok, scalar2=None,
                                        op0=ALU.mult)
                nc.vector.tensor_tensor(out=scale_eff, in0=scale_eff,
                                        in1=sc_sb, op=ALU.add)

            deferred = []   # (stage_tile, oi, tb) awaiting scale_eff

            with tc.tile_pool(name="ps", bufs=8, space="PSUM") as psp:
                for s in range(cfg.nslab):
                    ostart = s * cfg.oslab

                    # ---- w slab load (512-row quads) + quantize (pairs) ----
                    wq_tiles = []
                    for kb in range(cfg.kc4):
                        wr = wst.tile([128, 4, cfg.oslab], F32, name="wr")
                        nc.gpsimd.dma_start(
                            out=wr,
                            in_=wT[kb * 512:(kb + 1) * 512,
                                   ostart:ostart + cfg.oslab]
                            .rearrange("(four p) o -> p four o", p=128))
                        ga = qap.tile([128, 4, cfg.oslab], F32, name="ga")
                        nc.vector.tensor_scalar(
                            out=ga, in0=wr, scalar1=2.0, scalar2=-1.25,
                            op0=ALU.mult, op1=ALU.max)
                        gb = qbp.tile([128, 4, cfg.oslab], I32, name="gb")
                        nc.vector.tensor_scalar(
                            out=gb, in0=ga, scalar1=1.25, scalar2=None,
                            op0=ALU.min)
                        for half in range(2):
                            wq_t = wqp.tile([128, 2, cfg.oslab], F8,
                                            name="wq_t")
                            nc.vector.tensor_copy(
                                out=wq_t, in_=gb[:, 2 * half:2 * half + 2, :])
                            wq_tiles.append(wq_t)
                        if s == 1:
                            # second half of the token sums, woven through
                            # the DVE stream once all x pairs have landed
                            for i in range(2):
                                ib = cfg.kc // 2 + 2 * kb + i
                                nc.vector.reduce_sum(
                                    out=sums[:, ib:ib + 1],
                                    in_=xp[ib // 2][:, ib % 2, :],
                                    axis=mybir.AxisListType.X)
                    if s == 1:
                        nc.sync.dma_start(out=cc_in, in_=sums)
                        nc.gpsimd.collective_compute(
                            "AllReduce", ALU.add,
                            replica_groups=[list(range(cfg.ncores))],
                            ins=[cc_in.opt()], outs=[cc_out.opt()],
                        )
                    if s == cfg.direct_slab:
                        # Tile deps are program-order: scale_eff's producers
                        # must be emitted before any consumer. By the time
                        # the DVE reaches these ops (wq-pool gated, ~3 slabs
                        # behind s) cc_out has landed even at worst-case
                        # launch skew, so the wait never starves quantize.
                        emit_lrls()
                    if s >= cfg.direct_slab:
                        # fix up one staged slab (scratch -> scale -> yT),
                        # emitted BEFORE this slab's own epilogues so the
                        # last fixup drains under the final slab's matmuls.
                        # Rides sync (loads/stores) + DVE (in-place scale
                        # mult) to keep ACT free for bank-freeing epilogues.
                        for sg, oi, tb in deferred[(s - cfg.direct_slab) * 8:
                                                   (s - cfg.direct_slab + 1) * 8]:
                            ft = ftp.tile([128, cfg.tblk], F32, name="ft")
                            nc.sync.dma_start(out=ft, in_=sg)
                            nc.vector.tensor_tensor(
                                out=ft, in0=ft,
                                in1=scale_eff[:, oi:oi + 1]
                                .broadcast_to((128, cfg.tblk)),
                                op=ALU.mult)
                            nc.sync.dma_start(
                                out=yT[oi * 128:(oi + 1) * 128,
                                       tb * cfg.tblk:(tb + 1) * cfg.tblk],
                                in_=ft)

                    # ---- matmuls: 256-deep DoubleRow chunks. The last slab
                    # runs group-major so each group's epilogue drains under
                    # the remaining groups' matmuls instead of after them ----
                    pst = [[psp.tile([128, cfg.tblk], F32, name="pst")
                            for _ in range(cfg.ntblk)]
                           for _ in range(cfg.nos)]
                    if s == cfg.nslab - 1:
                        order = [(c, os_, tb)
                                 for os_ in range(cfg.nos)
                                 for tb in range(cfg.ntblk)
                                 for c in range(cfg.kc2)]
                    else:
                        order = [(c, os_, tb)
                                 for c in range(cfg.kc2)
                                 for os_ in range(cfg.nos)
                                 for tb in range(cfg.ntblk)]
                    for c, os_, tb in order:
                        nc.tensor.matmul(
                            pst[os_][tb],
                            lhsT=wq_tiles[c][:, :,
                                             os_ * 128:(os_ + 1) * 128],
                            rhs=xp[c][:, :,
                                      tb * cfg.tblk:(tb + 1) * cfg.tblk],
                            start=(c == 0), stop=(c == cfg.kc2 - 1),
                            perf_mode=DR,
                        )

                    # ---- epilogues ----
                    # Slabs < direct_slab: scale_eff may not exist yet
                    # (launch skew puts cc_out anywhere in 130..310 us), so
                    # write UNSCALED y to DRAM scratch — the bank-freeing
                    # path has no scale dependency at all. Each such slab is
                    # fixed up (read back, scale, store) direct_slab slabs
                    # later, fully overlapped under remaining PE work.
                    for os_ in range(cfg.nos):
                        oi = ostart // 128 + os_
                        for tb in range(cfg.ntblk):
                            ytt = ytp.tile([128, cfg.tblk], F32, name="ytt")
                            if s < cfg.direct_slab:
                                nc.scalar.activation(
                                    out=ytt, in_=pst[os_][tb],
                                    func=ACTF.Copy)
                                sg = cdram.tile([128, cfg.tblk], F32,
                                                name="scr")
                                nc.scalar.dma_start(out=sg, in_=ytt)
                                deferred.append((sg, oi, tb))
                            else:
                                nc.scalar.activation(
                                    out=ytt, in_=pst[os_][tb],
                                    func=ACTF.Copy,
                                    scale=scale_eff[:, oi:oi + 1])
                                nc.scalar.dma_start(
                                    out=yT[oi * 128:(oi + 1) * 128,
                                           tb * cfg.tblk:(tb + 1) * cfg.tblk],
                                    in_=ytt)



    if compile:
        nc.compile()
    return nc


def prep_inputs(cfg: Cfg, x, weight, scale, lrls_A, lrls_B):
    """Host-side sharding/layout marshalling + the fp8 input cast."""
    x_flat = np.ascontiguousarray(x.reshape(cfg.tok, cfg.din))
    x8 = x_flat.astype(ml_dtypes.float8_e4m3)         # RNE
    x8T_full = np.ascontiguousarray(x8.T)             # [din, tok] fp8
    wT = np.ascontiguousarray(weight.T)               # [din, dout]
    b_pk = np.ascontiguousarray(
        lrls_B.reshape(cfg.r, cfg.kc, 128).transpose(0, 2, 1).reshape(
            cfg.r, cfg.din))
    a_p = np.ascontiguousarray(
        lrls_A.reshape(cfg.oc, 128, cfg.r).transpose(1, 0, 2).reshape(
            128, cfg.oc * cfg.r))
    scale_pc = np.ascontiguousarray(scale.reshape(cfg.oc, 128).T)

    in_maps = []
    for c in range(cfg.ncores):
        x8T_c = np.ascontiguousarray(
            x8T_full[:, c * cfg.tsh:(c + 1) * cfg.tsh])
        in_maps.append({"x8T": x8T_c, "wT": wT, "scale_pc": scale_pc,
                        "b_pk": b_pk, "a_p": a_p})
    return in_maps


def assemble_output(cfg: Cfg, results, out_shape):
    y_flat = np.empty((cfg.tok, cfg.dout), np.float32)
    for c in range(cfg.ncores):
        y_flat[c * cfg.tsh:(c + 1) * cfg.tsh, :] = results[c]["yT"].T
    return y_flat.reshape(out_shape)


_NC_CACHE = {}


def run(cfg: Cfg, x, weight, scale, lrls_A, lrls_B, out_shape, **run_kwargs):
    key = (cfg.tok, cfg.din, cfg.dout, cfg.tsh, cfg.oslab, cfg.tblk,
           cfg.direct_slab)
    if key not in _NC_CACHE:
        _NC_CACHE[key] = build(cfg)
    nc = _NC_CACHE[key]
    in_maps = prep_inputs(cfg, x, weight, scale, lrls_A, lrls_B)
    res = run_bass_kernel_spmd(nc, in_maps, core_ids=list(range(cfg.ncores)),
                               **run_kwargs)
    y = assemble_output(cfg, res.results, out_shape)
    return y, res


def kernel(x, weight, threshold, scale, lrls_A, lrls_B):
    # threshold input is unused: the reference hardcodes THRESH=0.5
    # (TrainState.threshold() at step 0), so the ternary cut sits at |w|=0.25.
    cfg = Cfg()
    x = np.asarray(x, np.float32)
    y, _ = run(cfg, x, np.asarray(weight, np.float32),
               np.asarray(scale, np.float32), np.asarray(lrls_A, np.float32),
               np.asarray(lrls_B, np.float32),
               out_shape=(x.shape[0], x.shape[1], np.asarray(weight).shape[0]))
    return y.astype(np.float32)
